# revision 57
# baseline (speedup 1.0000x reference)
"""HaarWavelet2D (level=2) Trainium2 kernel — fused-DVE redesign.

Contract: kernel(x, level) with x [8, 64, 256, 256] fp32, level=2.
Returns (low_freq, high_freq), each [8, 64, 256, 256] fp32 — matching the
jax reference (2-level Haar decomposition with bilinear resizes).

Sharding: data-parallel over batch — core b processes x[b] (64 channels),
in 16 blocks of 4 channels (2 sub-iterations of G=2 for the PSUM-bound
matmul stages, block-wide elementwise stages to amortize per-op fixed
costs). Layout: row-pairs in partitions (partition p holds rows 2p/2p+1
as a parity dim), (channel, parity, col) in the free dims, so every HBM
transfer moves 2KB-contiguous chunks; loads are software-pipelined one
block ahead of the stores on the sync queue.

Engine placement:
  DVE    : s = pair-sum / |d| (fused absdiff, straight from f32),
           max/ch0/ch1 assembly, fused Idx/PageIdx-blend horizontal
           resizes (255->256 in one paged call, 128->256 interleaved)
  Tensor : row-diff t1 via +/-identity matmuls, all vertical resizes
           (parity-split banded matrices, bf16, one packed weight DMA)
  Scalar : |0.5*t1| from PSUM, PSUM->SBUF bf16 evacuations + pad cols
  DMA    : 1 load (f32) + 1 partition-shift copy per sub, 2 bf16
           stores per block
Outputs are written bf16 to HBM and upcast on the host (halves store
traffic; validated rel err ~4e-3 vs the 2e-2 gate in model.py).
"""

import sys

if "/opt/trn_rl_repo" not in sys.path:
    sys.path.insert(0, "/opt/trn_rl_repo")

import numpy as np
import ml_dtypes

BF = ml_dtypes.bfloat16

B_, C_, H_, W_ = 8, 64, 256, 256
NCORES = 8
G = 2  # channels per inner iteration


# ----------------------------------------------------------------------------
# custom fused DVE ops (registered into dve_ops.OPS at import)
# ----------------------------------------------------------------------------

_DVE_OPS = None


def _register_dve_ops():
    global _DVE_OPS
    if _DVE_OPS is not None:
        return _DVE_OPS
    from concourse.dve_spec import (
        Spec, Src0, Src1, C0, C1, Idx, Zero, PageIdx, maxx, scan, lower, AluOp,
    )
    from concourse import dve_ops as DO
    from concourse.dve_uop import DveOpSpec

    def mk(name, spec, subdim=False):
        for op in DO.OPS:
            if op.name == name:
                return op
        shas = {}
        for ver in ("v3", "v4"):
            try:
                uops = lower(spec, ver=ver)
                shas[ver] = DveOpSpec(
                    name=name, opcode=0, uops=uops, rd1_en=True
                ).sha(ver)
            except Exception:
                pass
        op = DO.DveOp(name, spec, subdim=subdim, uops_sha=shas)
        DO.OPS.append(op)
        DO.CUSTOM_DVE_SPECS[op.name] = op.spec
        DO._SUB_OPCODE_FOR_NAME[op.name] = (
            DO._CUSTOM_DVE_ROW_BASE + len(DO.OPS) - 1)
        assert DO._SUB_OPCODE_FOR_NAME[op.name] < 0x20
        return op

    # out = |in0 - in1| * s0
    absdiff = mk(
        "HW19_ABSDIFF",
        Spec(body=maxx(Src0 - Src1, Src1 - Src0) * C0,
             reference=lambda in0, in1, s0, s1, imm2: np.abs(in0 - in1) * s0))
    # out = in1 + (s0 + Idx*s1) * (in0 - in1)   (linear-in-index blend)
    blend = mk(
        "HW19_BLEND",
        Spec(body=Src1 + (C0 + Idx * C1) * (Src0 - Src1),
             reference=lambda in0, in1, s0, s1, imm2: in1 + (
                 s0 + np.arange(in0.shape[-1]) * s1) * (in0 - in1)))
    # paged 255->256 resize: w = (k_in_page + 1.5)/256 with s0 = 1/512 and
    # s1 = -255/256 cancelling the global-Idx page offset
    def _blendpg_ref(in0, in1, s0, s1, imm2):
        S, N = in0.shape[-2], in0.shape[-1]
        k = np.arange(S * N).reshape(S, N)
        w = s0 + (k + 1) * 2 * s0 + np.arange(S)[:, None] * s1
        return in1 + w * (in0 - in1)
    blendpg = mk(
        "HW19_BLENDPG",
        Spec(body=Src1 + (scan(AluOp.ADD, C0 + C0, init=C0)
                          + PageIdx(Zero, C1)) * (Src0 - Src1),
             reference=_blendpg_ref),
        subdim=True)
    _DVE_OPS = (absdiff, blend, blendpg)
    return _DVE_OPS


# ----------------------------------------------------------------------------
# host-side weight construction
# ----------------------------------------------------------------------------

def _resize_matrix(n, N):
    M = np.zeros((N, n), dtype=np.float64)
    for i in range(N):
        c = (i + 0.5) * n / N - 0.5
        j0 = int(np.floor(c))
        f = c - j0
        M[i, min(max(j0, 0), n - 1)] += 1.0 - f
        M[i, min(max(j0 + 1, 0), n - 1)] += f
    return M


def _build_weights():
    V255 = _resize_matrix(255, 256)
    V128 = _resize_matrix(128, 256)
    Sv1 = np.zeros((255, 256))
    for r in range(255):
        Sv1[r, r] = 1.0
        Sv1[r, r + 1] = 1.0
    Va = 0.25 * (V255 @ Sv1)      # [256, 256] vertical resize+pair-sum fold
    V255s = 0.25 * V255           # [256, 255]
    V128q = 0.25 * V128           # [256, 128]

    w = {}
    for par in (0, 1):
        # lhsT convention: out[po,f] = sum_pi lhsT[pi,po] * rhs[pi,f]
        w[f"w_va_{par}e"] = Va[par::2, 0::2].T        # [128,128]
        w[f"w_va_{par}o"] = Va[par::2, 1::2].T
        w[f"w_vh_{par}e"] = V255s[par::2, 0::2].T     # [128,128]
        w[f"w_vh_{par}o"] = V255s[par::2, 1::2].T     # [127,128]
        w[f"w_vq_{par}"] = V128q[par::2, :].T         # [128,128]
    w["w_id"] = np.eye(128)
    w["w_idn"] = -np.eye(128)
    # t1O[po] = sO[po] - sE[po+1]:  lhsT[pi,po] = -1 iff pi == po+1
    w["w_shn"] = np.diag(-np.ones(127), k=-1)
    # adE2[po] = adE[po+1]: positive partition shift
    w["w_shp"] = np.diag(np.ones(127), k=-1)
    return {k: v.astype(BF) for k, v in w.items()}


# fixed packing order for the single-DMA weight tile; [127,128] weights are
# zero-row-padded (sliced back to 127 partitions at the call site)
_W_ORDER = ["w_va_0e", "w_va_0o", "w_va_1e", "w_va_1o",
            "w_vh_0e", "w_vh_0o", "w_vh_1e", "w_vh_1o",
            "w_vq_0", "w_vq_1", "w_id", "w_idn", "w_shn", "w_shp"]


def _pack_weights(w):
    cols = []
    for k in _W_ORDER:
        a = w[k]
        if a.shape[0] < 128:
            a = np.concatenate(
                [a, np.zeros((128 - a.shape[0], a.shape[1]), dtype=a.dtype)])
        cols.append(a)
    return np.stack(cols, axis=1).reshape(128, -1)  # [128, 13*128]


_WEIGHTS = None


def _weights():
    """Device-side operand dict: one packed weight tensor."""
    global _WEIGHTS
    if _WEIGHTS is None:
        _WEIGHTS = {"w_all": _pack_weights(_build_weights())}
    return _WEIGHTS


# ----------------------------------------------------------------------------
# bass program
# ----------------------------------------------------------------------------

_NC_CACHE = {}


def build_nc(C=C_):
    if C in _NC_CACHE:
        return _NC_CACHE[C]

    import concourse.bass as bass  # noqa: F401
    import concourse.bacc as bacc
    import concourse.tile as tile
    import concourse.mybir as mybir

    absdiff_op, blend_op, blendpg_op = _register_dve_ops()

    F32 = mybir.dt.float32
    BF16 = mybir.dt.bfloat16
    Alu = mybir.AluOpType
    Act = mybir.ActivationFunctionType
    P = 128

    nc = bacc.Bacc("TRN2", target_bir_lowering=False)
    x_d = nc.dram_tensor("x", [C, H_, W_], F32, kind="ExternalInput")
    nw = len(_W_ORDER)
    w_d = nc.dram_tensor("w_all", [128, nw * 128], BF16, kind="ExternalInput")
    raw_w = _build_weights()
    low_d = nc.dram_tensor("low", [C, H_, W_], BF16, kind="ExternalOutput")
    high_d = nc.dram_tensor("high", [C, H_, W_], BF16, kind="ExternalOutput")

    with tile.TileContext(nc) as tc:
        with (
            tc.tile_pool(name="consts", bufs=1) as consts,
            tc.tile_pool(name="xin", bufs=3) as xin,
            tc.tile_pool(name="sd", bufs=3) as sd,
            tc.tile_pool(name="mid", bufs=3) as mid,
            tc.tile_pool(name="hor", bufs=2) as hor,
            tc.tile_pool(name="lv1", bufs=2) as lv1,
            tc.tile_pool(name="outp", bufs=2) as outp,
            tc.tile_pool(name="pst1", bufs=1, space="PSUM") as pst1,
            tc.tile_pool(name="psL", bufs=1, space="PSUM") as psL,
            tc.tile_pool(name="psh", bufs=1, space="PSUM") as psh,
            tc.tile_pool(name="ps1", bufs=1, space="PSUM") as ps1,
        ):
            def load_weights():
                w_sb = consts.tile([128, nw, 128], BF16, tag="w_all")
                nc.sync.dma_start(
                    out=w_sb,
                    in_=w_d[:, :].rearrange("p (n w) -> p n w", w=128))
                wt = {}
                for i, name in enumerate(_W_ORDER):
                    rows = raw_w[name].shape[0]
                    wt[name] = w_sb[0:rows, i, :]
                return wt

            def blend255(qf, outf, name):
                """qf: [P, 4, 256] bf16 (col 255 = pad copy of 254).
                outf: [P, 4, 256]; out[i] = q[i] + w_i*(q[i-1]-q[i]),
                w_i = (i+0.5)/256; out[0] = q[0] (edge)."""
                nc.vector._custom_dve(
                    blendpg_op,
                    out=outf[:, :, 1:256],
                    in0=qf[:, :, 0:255],
                    in1=qf[:, :, 1:256],
                    s0=1.0 / 512.0,
                    s1=-255.0 / 256.0,
                )
                nc.scalar.copy(out=outf[:, :, 0:1], in_=qf[:, :, 0:1])

            def blend128(qf, outf):
                """qf: [P, 4, 128]; outf: [P, 4, 256] interleaved 2x
                upsample: out[2k]=.75q[k]+.25q[k-1], out[2k+1]=.75q[k]+.25q[k+1]."""
                nc.vector._custom_dve(
                    blend_op,
                    out=outf[:, :, 2:256:2],
                    in0=qf[:, :, 0:127],
                    in1=qf[:, :, 1:128],
                    s0=0.25,
                    s1=0.0,
                )
                nc.vector._custom_dve(
                    blend_op,
                    out=outf[:, :, 1:254:2],
                    in0=qf[:, :, 1:128],
                    in1=qf[:, :, 0:127],
                    s0=0.25,
                    s1=0.0,
                )
                nc.scalar.copy(out=outf[:, :, 0:1], in_=qf[:, :, 0:1])
                nc.scalar.copy(out=outf[:, :, 255:256],
                                      in_=qf[:, :, 127:128])

            # 2-iteration blocks: matmul/PSUM stages run per sub-iteration
            # (G=2 channels), elementwise stages run once per block (C4=4
            # channels) on doubled tiles to amortize per-op fixed costs.
            C4 = 2 * G
            n_blk = C // C4

            # software-pipelined loads: block k+1's loads are issued on the
            # sync queue BEFORE block k's stores, so the store's wait on the
            # block's final vector ops doesn't delay the next block's input
            pending = {}

            def issue_loads(blk):
                t = xin.tile([P, C4, 2, 256], F32, tag="xC")
                nc.sync.dma_start(
                    out=t,
                    in_=x_d[blk * C4:(blk + 1) * C4, :, :].rearrange(
                        "c (p r) w -> p c r w", r=2))
                pending[blk] = t

            # first input load beats the weight load onto the sync queue:
            # weights aren't needed until the first matmul
            issue_loads(0)
            wtile = load_weights()
            for blk in range(n_blk):
                c0b = blk * C4
                if blk + 1 < n_blk:
                    issue_loads(blk + 1)
                # block tiles: pages ordered (channel, row-parity)
                qLb = hor.tile([P, 2 * C4, 256], BF16, tag="qLb")
                qhb = hor.tile([P, 2 * C4, 256], BF16, tag="qhb")
                L0 = hor.tile([P, 2 * C4, 256], BF16, tag="L0")
                h0 = hor.tile([P, 2 * C4, 256], BF16, tag="h0")

                # ---- level-0 horizontal pair sum / |diff| (block-wide) -----
                xCb = pending.pop(blk)
                xCbf = xCb.rearrange("p c r w -> p (c r) w")   # [P, 8, 256]
                # 256-element page pitch keeps every page 4B-aligned
                # (255-pitch puts odd pages at a 2B offset -> 1x DVE mode)
                s_blk = sd.tile([P, 2 * C4, 256], BF16, tag="s")
                nc.vector.tensor_tensor(
                    out=s_blk[:, :, 0:255],
                    in0=xCbf[:, :, 0:255], in1=xCbf[:, :, 1:256],
                    op=Alu.add)
                ad_blk = sd.tile([P, 2 * C4, 256], BF16, tag="ad")
                nc.vector._custom_dve(
                    absdiff_op, out=ad_blk[:, :, 0:255],
                    in0=xCbf[:, :, 0:255], in1=xCbf[:, :, 1:256], s0=1.0)
                sv_b = s_blk.rearrange("p (c r) w -> p c r w", r=2)
                adv_b = ad_blk.rearrange("p (c r) w -> p c r w", r=2)

                for sub in (0, 1):
                    pg0 = sub * 2 * G           # first (c,r) page of this sub
                    cs = slice(sub * G, (sub + 1) * G)
                    sE = sv_b[:, cs, 0, 0:255]                # [P, G, 255]
                    sO = sv_b[:, cs, 1, 0:255]
                    adE = adv_b[:, cs, 0, 0:255]
                    adO = adv_b[:, cs, 1, 0:255]

                    # ---- t1 = row-diff of s, on the tensor engine ----------
                    t1EO = pst1.tile([P, 2, 512], F32, tag="t1EO")
                    nc.tensor.matmul(out=t1EO[:, 0, 0:510],
                                     lhsT=wtile["w_id"][:, :], rhs=sE,
                                     start=True, stop=False)
                    nc.tensor.matmul(out=t1EO[:, 0, 0:510],
                                     lhsT=wtile["w_idn"][:, :], rhs=sO,
                                     start=False, stop=True)
                    nc.tensor.matmul(out=t1EO[:, 1, 0:510],
                                     lhsT=wtile["w_id"][:, :], rhs=sO,
                                     start=True, stop=False)
                    nc.tensor.matmul(out=t1EO[:, 1, 0:510],
                                     lhsT=wtile["w_shn"][:, :], rhs=sE,
                                     start=False, stop=True)
                    a1 = mid.tile([P, 2, 510], BF16, tag=f"a1{sub}")
                    nc.scalar.activation(out=a1, in_=t1EO[:, :, 0:510],
                                         func=Act.Abs, scale=0.5)
                    a1E = a1[:, 0].rearrange("p (c w) -> p c w", w=255)
                    a1O = a1[:, 1].rearrange("p (c w) -> p c w", w=255)

                    # partition-shift |d| via the tensor engine into the t1
                    # buffer after the scalar abs has read it (no DMA latency
                    # on the mO chain); scalar evacuates to bf16 so the mO
                    # max runs in the 2x DVE mode
                    t1b = pst1.tile([P, 2, 512], F32, tag="t1EO")
                    nc.tensor.matmul(out=t1b[:, 0, 0:510],
                                     lhsT=wtile["w_shp"][:, :], rhs=adE,
                                     start=True, stop=True)
                    adE2t = mid.tile([127, G, 256], BF16,
                                     tag=f"adE2{sub}")
                    nc.scalar.copy(
                        out=adE2t[:, :, 0:255],
                        in_=t1b[0:127, 0, 0:510].rearrange(
                            "p (c w) -> p c w", w=255))
                    adE2 = adE2t[:, :, 0:255]

                    # ---- m = max(|d|,|d'|) ---------------------------------
                    # ch0 = a1 + m is folded into the Y_h matmul accumulation
                    mEt = mid.tile([P, G, 256], BF16, tag=f"mE{sub}")
                    mOt = mid.tile([127, G, 256], BF16, tag=f"mO{sub}")
                    mE = mEt[:, :, 0:255]
                    mO = mOt[:, :, 0:255]
                    nc.vector.tensor_tensor(out=mE, in0=adE, in1=adO,
                                            op=Alu.max)
                    nc.vector.tensor_tensor(out=mO, in0=adO[0:127], in1=adE2,
                                            op=Alu.max)

                    # ---- level-0 vertical matmuls (parity-split) -----------
                    Y_L = psL.tile([P, 2, 512], F32, tag="Y_L")
                    Y_h = psh.tile([P, 2, 512], F32, tag="Y_h")
                    for par in (0, 1):
                        nc.tensor.matmul(out=Y_L[:, par, 0:510],
                                         lhsT=wtile[f"w_va_{par}e"][:, :],
                                         rhs=sE, start=True, stop=False)
                        nc.tensor.matmul(out=Y_L[:, par, 0:510],
                                         lhsT=wtile[f"w_va_{par}o"][:, :],
                                         rhs=sO, start=False, stop=True)
                        nc.tensor.matmul(out=Y_h[:, par, 0:510],
                                         lhsT=wtile[f"w_vh_{par}e"][:, :],
                                         rhs=a1E, start=True, stop=False)
                        nc.tensor.matmul(out=Y_h[:, par, 0:510],
                                         lhsT=wtile[f"w_vh_{par}e"][:, :],
                                         rhs=mE, start=False, stop=False)
                        nc.tensor.matmul(out=Y_h[:, par, 0:510],
                                         lhsT=wtile[f"w_vh_{par}o"][:, :],
                                         rhs=a1O[0:127], start=False,
                                         stop=False)
                        nc.tensor.matmul(out=Y_h[:, par, 0:510],
                                         lhsT=wtile[f"w_vh_{par}o"][:, :],
                                         rhs=mO, start=False, stop=True)

                    # ---- evac to bf16 block-tile quarters + pad col --------
                    # (pad reads PSUM directly — independent of the main copy)
                    for q, Y in ((qLb, Y_L), (qhb, Y_h)):
                        qq = q[:, pg0:pg0 + 2 * G, :].rearrange(
                            "p (c r) w -> p c r w", r=2)
                        Yv = Y[:, :, 0:510].rearrange(
                            "p r (c w) -> p c r w", w=255)
                        nc.scalar.copy(out=qq[:, :, :, 0:255], in_=Yv)
                        nc.scalar.copy(out=qq[:, :, :, 255:256],
                                       in_=Yv[:, :, :, 254:255])

                # ---- level-0 horizontal resizes (block-wide) ---------------
                blend255(qLb, L0, "L0")
                blend255(qhb, h0, "h0")

                # ---- level-1 elementwise (block-wide) ----------------------
                s2 = lv1.tile([P, 2 * C4, 128], BF16, tag="s2")
                nc.vector.tensor_tensor(out=s2, in0=L0[:, :, 0:256:2],
                                        in1=L0[:, :, 1:256:2], op=Alu.add)
                ad2 = lv1.tile([P, 2 * C4, 128], BF16, tag="ad2")
                nc.vector._custom_dve(
                    absdiff_op, out=ad2,
                    in0=L0[:, :, 0:256:2], in1=L0[:, :, 1:256:2], s0=1.0)
                s2v = s2.rearrange("p (c r) w -> p c r w", r=2)
                ad2v = ad2.rearrange("p (c r) w -> p c r w", r=2)
                t1c = lv1.tile([P, C4, 128], BF16, tag="t1c")
                nc.vector.tensor_tensor(out=t1c, in0=s2v[:, :, 0],
                                        in1=s2v[:, :, 1], op=Alu.subtract)
                a1b = lv1.tile([P, C4, 128], BF16, tag="a1b")
                nc.scalar.activation(out=a1b, in_=t1c, func=Act.Abs, scale=0.5)
                m1 = lv1.tile([P, C4, 128], BF16, tag="m1")
                nc.vector.tensor_tensor(out=m1, in0=ad2v[:, :, 0],
                                        in1=ad2v[:, :, 1], op=Alu.max)

                # ---- level-1 vertical matmuls + evac (per sub, PSUM) -------
                # lsum1 = s2e+s2o and ch1 = a1b+m1 are folded into the
                # matmul accumulations (weights reused -> no extra LDWEIGHTS)
                # qq block tile: pages 0..7 = low (c,r), 8..15 = h1 (c,r)
                qq = lv1.tile([P, 4 * C4, 128], BF16, tag="qq")
                for sub in (0, 1):
                    Y_lo = ps1.tile([P, 2, G, 128], F32, tag="Y_lo")
                    Y_h1 = ps1.tile([P, 2, G, 128], F32, tag="Y_h1")
                    sl = slice(sub * G, (sub + 1) * G)
                    for par in (0, 1):
                        nc.tensor.matmul(out=Y_lo[:, par],
                                         lhsT=wtile[f"w_vq_{par}"][:, :],
                                         rhs=s2v[:, sl, 0], start=True,
                                         stop=False)
                        nc.tensor.matmul(out=Y_lo[:, par],
                                         lhsT=wtile[f"w_vq_{par}"][:, :],
                                         rhs=s2v[:, sl, 1], start=False,
                                         stop=True)
                        nc.tensor.matmul(out=Y_h1[:, par],
                                         lhsT=wtile[f"w_vq_{par}"][:, :],
                                         rhs=a1b[:, sl], start=True,
                                         stop=False)
                        nc.tensor.matmul(out=Y_h1[:, par],
                                         lhsT=wtile[f"w_vq_{par}"][:, :],
                                         rhs=m1[:, sl], start=False,
                                         stop=True)
                    nc.scalar.copy(
                        out=qq[:, sub * 2 * G:(sub + 1) * 2 * G, :].rearrange(
                            "p (c r) w -> p c r w", r=2),
                        in_=Y_lo.rearrange("p r c w -> p c r w"))
                    nc.scalar.copy(
                        out=qq[:, 2 * C4 + sub * 2 * G:
                               2 * C4 + (sub + 1) * 2 * G, :].rearrange(
                            "p (c r) w -> p c r w", r=2),
                        in_=Y_h1.rearrange("p r c w -> p c r w"))

                # ---- level-1 horizontal (2x upsample, low+h1 together) -----
                lowhi = outp.tile([P, 4 * C4, 256], BF16, tag="lowhi")
                blend128(qq, lowhi)
                # high = h0 + h1 via identity-matmul accumulation (tensor has
                # slack; scalar evacuates) — two 2-bank halves through the
                # freed Y_h PSUM buffer
                highI = outp.tile([P, 2 * C4, 256], BF16, tag="highI")
                h1I = lowhi[:, 2 * C4:4 * C4, :]
                if blk == n_blk - 1:
                    # last block: direct vector add — skips the tensor+scalar
                    # round trip that would otherwise gate the final stores
                    nc.vector.tensor_tensor(out=highI, in0=h0, in1=h1I,
                                            op=Alu.add)
                else:
                    for half in (0, 1):
                        ph = psh.tile([P, 2, 512], F32, tag="Y_h")
                        hs = slice(half * C4, (half + 1) * C4)
                        for q in (0, 1):
                            qs = slice(half * C4 + q * 2,
                                       half * C4 + q * 2 + 2)
                            nc.tensor.matmul(
                                out=ph[:, q, 0:512],
                                lhsT=wtile["w_id"][:, :],
                                rhs=h0[:, qs, :], start=True, stop=False)
                            nc.tensor.matmul(
                                out=ph[:, q, 0:512],
                                lhsT=wtile["w_id"][:, :],
                                rhs=h1I[:, qs, :], start=False, stop=True)
                        nc.scalar.copy(
                            out=highI[:, hs, :],
                            in_=ph.rearrange("p a w -> p (a w)")[:, 0:1024]
                            .rearrange("p (c w) -> p c w", w=256))

                # ---- store (bf16, row-pair chunks) -------------------------
                nc.sync.dma_start(
                    out=low_d[c0b:c0b + C4, :, :].rearrange(
                        "c (p r) w -> p c r w", r=2),
                    in_=lowhi[:, 0:2 * C4, :].rearrange(
                        "p (c r) w -> p c r w", r=2))
                nc.sync.dma_start(
                    out=high_d[c0b:c0b + C4, :, :].rearrange(
                        "c (p r) w -> p c r w", r=2),
                    in_=highI.rearrange("p (c r) w -> p c r w", r=2))

    nc.compile()
    _NC_CACHE[C] = nc
    return nc


# ----------------------------------------------------------------------------
# host entry points
# ----------------------------------------------------------------------------

_RUNNER = None


def _get_runner():
    """Builds (once) a cached sharded jit executable over the 8 cores."""
    global _RUNNER
    if _RUNNER is not None:
        return _RUNNER

    import jax
    from jax.sharding import Mesh, PartitionSpec, NamedSharding
    from jax.experimental.shard_map import shard_map
    import concourse.mybir as mybir
    from concourse import bass2jax
    from concourse.bass2jax import _bass_exec_p, partition_id_tensor

    bass2jax.install_neuronx_cc_hook()
    nc = build_nc(C_)

    partition_name = nc.partition_id_tensor.name if nc.partition_id_tensor else None
    in_names, out_names, out_avals = [], [], []
    for alloc in nc.m.functions[0].allocations:
        if not isinstance(alloc, mybir.MemoryLocationSet):
            continue
        name = alloc.memorylocations[0].name
        if alloc.kind == "ExternalInput":
            if name != partition_name:
                in_names.append(name)
        elif alloc.kind == "ExternalOutput":
            out_names.append(name)
            out_avals.append(jax.core.ShapedArray(
                tuple(alloc.tensor_shape), mybir.dt.np(alloc.dtype)))
    n_params = len(in_names)
    all_in_names = list(in_names) + list(out_names)
    if partition_name is not None:
        all_in_names.append(partition_name)

    def _body(*args):
        operands = list(args)
        if partition_name is not None:
            operands.append(partition_id_tensor())
        return tuple(_bass_exec_p.bind(
            *operands,
            out_avals=tuple(out_avals),
            in_names=tuple(all_in_names),
            out_names=tuple(out_names),
            lowering_input_output_aliases=(),
            sim_require_finite=True,
            sim_require_nnan=True,
            nc=nc,
        ))

    devices = jax.devices()[:NCORES]
    mesh = Mesh(np.asarray(devices), ("core",))
    n_in = n_params + len(out_names)
    sharded = jax.jit(shard_map(
        _body, mesh=mesh,
        in_specs=(PartitionSpec("core"),) * n_in,
        out_specs=(PartitionSpec("core"),) * len(out_names),
        check_rep=False))

    shard0 = NamedSharding(mesh, PartitionSpec("core"))
    wt = _weights()
    static = {}
    for name in in_names:
        if name == "x":
            continue
        arr = np.concatenate([wt[name]] * NCORES, axis=0)
        static[name] = jax.device_put(arr, shard0)
    for name, aval in zip(out_names, out_avals):
        z = np.zeros((aval.shape[0] * NCORES,) + tuple(aval.shape[1:]),
                     dtype=aval.dtype)
        static[name] = jax.device_put(z, shard0)

    def run(x_global):
        ops = []
        for name in in_names:
            ops.append(x_global if name == "x" else static[name])
        for name in out_names:
            ops.append(static[name])
        outs = sharded(*ops)
        return dict(zip(out_names, outs))

    _RUNNER = (run, shard0)
    return _RUNNER


def _run_device(x, trace=False):
    """x: [8, 64, 256, 256] fp32. Returns (low, high, results_obj)."""
    if trace:
        import shutil
        from concourse import bass_utils
        nc = build_nc(C_)
        wt = _weights()
        in_maps = [dict(wt, x=np.ascontiguousarray(x[b])) for b in range(NCORES)]
        shutil.rmtree("/tmp/bass_trace", ignore_errors=True)
        import os
        os.makedirs("/tmp/bass_trace", exist_ok=True)
        res = bass_utils.run_bass_kernel_spmd(
            nc, in_maps, core_ids=list(range(NCORES)), trace=True,
            tmpdir="/tmp/bass_trace")
        low = np.stack([np.asarray(res.results[b]["low"]) for b in range(NCORES)])
        high = np.stack([np.asarray(res.results[b]["high"]) for b in range(NCORES)])
        return low.astype(np.float32), high.astype(np.float32), res

    run, _ = _get_runner()
    outs = run(np.ascontiguousarray(x).reshape(B_ * C_, H_, W_))
    low = np.asarray(outs["low"]).reshape(B_, C_, H_, W_).astype(np.float32)
    high = np.asarray(outs["high"]).reshape(B_, C_, H_, W_).astype(np.float32)
    return low, high, None


def _fallback(x, level):
    """Numpy port of the reference for unexpected shapes/levels."""
    xl = x.astype(np.float64)
    Bb, Cc, H, W = xl.shape
    low = xl
    high = np.zeros_like(xl)

    def up(a, n_r, n_c):
        Mr = _resize_matrix(a.shape[-2], n_r)
        Mc = _resize_matrix(a.shape[-1], n_c)
        return np.einsum("ij,...jk,lk->...il", Mr, a, Mc)

    for lv in range(level):
        stride = 2 ** lv
        if H // stride < 2 or W // stride < 2:
            break
        x00 = low[..., 0:H - 1:stride, 0:W - 1:stride]
        x01 = low[..., 0:H - 1:stride, 1:W:stride]
        x10 = low[..., 1:H:stride, 0:W - 1:stride]
        x11 = low[..., 1:H:stride, 1:W:stride]
        ll = (x00 + x01 + x10 + x11) * 0.25
        lh = (x00 + x01 - x10 - x11) * 0.25
        hl = (x00 - x01 + x10 - x11) * 0.25
        hh = (x00 - x01 - x10 + x11) * 0.25
        ch = np.abs(lh) + np.abs(hl) + np.abs(hh)
        high = high + up(ch, H, W)
        low = up(ll, H, W)
    if level > 0:
        high = high / level
    return low.astype(np.float32), high.astype(np.float32)


def kernel(x, level):
    x = np.asarray(x, dtype=np.float32)
    level = int(level)
    if level != 2 or x.shape != (B_, C_, H_, W_):
        return _fallback(x, level)
    low, high, _ = _run_device(x)
    return low, high


# revision 59
# speedup vs baseline: 1.0078x; 1.0078x over previous
"""HaarWavelet2D (level=2) Trainium2 kernel — fused-DVE redesign.

Contract: kernel(x, level) with x [8, 64, 256, 256] fp32, level=2.
Returns (low_freq, high_freq), each [8, 64, 256, 256] fp32 — matching the
jax reference (2-level Haar decomposition with bilinear resizes).

Sharding: data-parallel over batch — core b processes x[b] (64 channels),
in 16 blocks of 4 channels (2 sub-iterations of G=2 for the PSUM-bound
matmul stages, block-wide elementwise stages to amortize per-op fixed
costs). Layout: row-pairs in partitions (partition p holds rows 2p/2p+1
as a parity dim), (channel, parity, col) in the free dims, so every HBM
transfer moves 2KB-contiguous chunks; loads are software-pipelined one
block ahead of the stores on the sync queue.

Engine placement:
  DVE    : s = pair-sum / |d| (fused absdiff, straight from f32),
           max/ch0/ch1 assembly, fused Idx/PageIdx-blend horizontal
           resizes (255->256 in one paged call, 128->256 interleaved)
  Tensor : row-diff t1 via +/-identity matmuls, all vertical resizes
           (parity-split banded matrices, bf16, one packed weight DMA)
  Scalar : |0.5*t1| from PSUM, PSUM->SBUF bf16 evacuations + pad cols
  DMA    : 1 load (f32) + 1 partition-shift copy per sub, 2 bf16
           stores per block
Outputs are written bf16 to HBM and upcast on the host (halves store
traffic; validated rel err ~4e-3 vs the 2e-2 gate in model.py).
"""

import sys

if "/opt/trn_rl_repo" not in sys.path:
    sys.path.insert(0, "/opt/trn_rl_repo")

import numpy as np
import ml_dtypes

BF = ml_dtypes.bfloat16

B_, C_, H_, W_ = 8, 64, 256, 256
NCORES = 8
G = 2  # channels per inner iteration


# ----------------------------------------------------------------------------
# custom fused DVE ops (registered into dve_ops.OPS at import)
# ----------------------------------------------------------------------------

_DVE_OPS = None


def _register_dve_ops():
    global _DVE_OPS
    if _DVE_OPS is not None:
        return _DVE_OPS
    from concourse.dve_spec import (
        Spec, Src0, Src1, C0, C1, Idx, Zero, One, PageIdx, maxx, scan, lower,
        AluOp,
    )
    from concourse import dve_ops as DO
    from concourse.dve_uop import DveOpSpec

    def mk(name, spec, subdim=False):
        for op in DO.OPS:
            if op.name == name:
                return op
        shas = {}
        for ver in ("v3", "v4"):
            try:
                uops = lower(spec, ver=ver)
                shas[ver] = DveOpSpec(
                    name=name, opcode=0, uops=uops, rd1_en=True
                ).sha(ver)
            except Exception:
                pass
        op = DO.DveOp(name, spec, subdim=subdim, uops_sha=shas)
        DO.OPS.append(op)
        DO.CUSTOM_DVE_SPECS[op.name] = op.spec
        DO._SUB_OPCODE_FOR_NAME[op.name] = (
            DO._CUSTOM_DVE_ROW_BASE + len(DO.OPS) - 1)
        assert DO._SUB_OPCODE_FOR_NAME[op.name] < 0x20
        return op

    # out = |in0 - in1| * s0
    absdiff = mk(
        "HW19_ABSDIFF",
        Spec(body=maxx(Src0 - Src1, Src1 - Src0) * C0,
             reference=lambda in0, in1, s0, s1, imm2: np.abs(in0 - in1) * s0))
    # out = in1 + (s0 + Idx*s1) * (in0 - in1)   (linear-in-index blend)
    blend = mk(
        "HW19_BLEND",
        Spec(body=Src1 + (C0 + Idx * C1) * (Src0 - Src1),
             reference=lambda in0, in1, s0, s1, imm2: in1 + (
                 s0 + np.arange(in0.shape[-1]) * s1) * (in0 - in1)))
    # paged 255->256 resize: w = (k_in_page + 1.5)/256 with s0 = 1/512 and
    # s1 = -255/256 cancelling the global-Idx page offset
    def _blendpg_ref(in0, in1, s0, s1, imm2):
        S, N = in0.shape[-2], in0.shape[-1]
        k = np.arange(S * N).reshape(S, N)
        w = s0 + (k + 1) * 2 * s0 + np.arange(S)[:, None] * s1
        return in1 + w * (in0 - in1)
    blendpg = mk(
        "HW19_BLENDPG",
        Spec(body=Src1 + (scan(AluOp.ADD, C0 + C0, init=C0)
                          + PageIdx(Zero, C1)) * (Src0 - Src1),
             reference=_blendpg_ref),
        subdim=True)
    # paged blend with independent slope/offset: w = s0*k_in_page + (s1+s0)
    # (page correction uses the hardware One: pages are 1/s0 elements wide)
    def _blendpg2_ref(in0, in1, s0, s1, imm2):
        S, N = in0.shape[-2], in0.shape[-1]
        k = np.arange(S * N).reshape(S, N)
        w = s1 + (k + 1) * s0 - np.arange(S)[:, None]
        return in1 + w * (in0 - in1)
    blendpg2 = mk(
        "HW19_BLENDPG2",
        Spec(body=Src1 + (scan(AluOp.ADD, C0, init=C1)
                          - PageIdx(Zero, One)) * (Src0 - Src1),
             reference=_blendpg2_ref),
        subdim=True)
    _DVE_OPS = (absdiff, blend, blendpg, blendpg2)
    return _DVE_OPS


# ----------------------------------------------------------------------------
# host-side weight construction
# ----------------------------------------------------------------------------

def _resize_matrix(n, N):
    M = np.zeros((N, n), dtype=np.float64)
    for i in range(N):
        c = (i + 0.5) * n / N - 0.5
        j0 = int(np.floor(c))
        f = c - j0
        M[i, min(max(j0, 0), n - 1)] += 1.0 - f
        M[i, min(max(j0 + 1, 0), n - 1)] += f
    return M


def _build_weights():
    V255 = _resize_matrix(255, 256)
    V128 = _resize_matrix(128, 256)
    Sv1 = np.zeros((255, 256))
    for r in range(255):
        Sv1[r, r] = 1.0
        Sv1[r, r + 1] = 1.0
    Va = 0.25 * (V255 @ Sv1)      # [256, 256] vertical resize+pair-sum fold
    V255s = 0.25 * V255           # [256, 255]
    V128q = 0.25 * V128           # [256, 128]

    w = {}
    for par in (0, 1):
        # lhsT convention: out[po,f] = sum_pi lhsT[pi,po] * rhs[pi,f]
        w[f"w_va_{par}e"] = Va[par::2, 0::2].T        # [128,128]
        w[f"w_va_{par}o"] = Va[par::2, 1::2].T
        w[f"w_vh_{par}e"] = V255s[par::2, 0::2].T     # [128,128]
        w[f"w_vh_{par}o"] = V255s[par::2, 1::2].T     # [127,128]
        w[f"w_vq_{par}"] = V128q[par::2, :].T         # [128,128]
    w["w_id"] = np.eye(128)
    w["w_idn"] = -np.eye(128)
    # t1O[po] = sO[po] - sE[po+1]:  lhsT[pi,po] = -1 iff pi == po+1
    w["w_shn"] = np.diag(-np.ones(127), k=-1)
    # adE2[po] = adE[po+1]: positive partition shift
    w["w_shp"] = np.diag(np.ones(127), k=-1)
    return {k: v.astype(BF) for k, v in w.items()}


# fixed packing order for the single-DMA weight tile; [127,128] weights are
# zero-row-padded (sliced back to 127 partitions at the call site)
_W_ORDER = ["w_va_0e", "w_va_0o", "w_va_1e", "w_va_1o",
            "w_vh_0e", "w_vh_0o", "w_vh_1e", "w_vh_1o",
            "w_vq_0", "w_vq_1", "w_id", "w_idn", "w_shn", "w_shp"]


def _pack_weights(w):
    cols = []
    for k in _W_ORDER:
        a = w[k]
        if a.shape[0] < 128:
            a = np.concatenate(
                [a, np.zeros((128 - a.shape[0], a.shape[1]), dtype=a.dtype)])
        cols.append(a)
    return np.stack(cols, axis=1).reshape(128, -1)  # [128, 13*128]


_WEIGHTS = None


def _weights():
    """Device-side operand dict: one packed weight tensor."""
    global _WEIGHTS
    if _WEIGHTS is None:
        _WEIGHTS = {"w_all": _pack_weights(_build_weights())}
    return _WEIGHTS


# ----------------------------------------------------------------------------
# bass program
# ----------------------------------------------------------------------------

_NC_CACHE = {}


def build_nc(C=C_):
    if C in _NC_CACHE:
        return _NC_CACHE[C]

    import concourse.bass as bass  # noqa: F401
    import concourse.bacc as bacc
    import concourse.tile as tile
    import concourse.mybir as mybir

    absdiff_op, blend_op, blendpg_op, blendpg2_op = _register_dve_ops()

    F32 = mybir.dt.float32
    BF16 = mybir.dt.bfloat16
    Alu = mybir.AluOpType
    Act = mybir.ActivationFunctionType
    P = 128

    nc = bacc.Bacc("TRN2", target_bir_lowering=False)
    x_d = nc.dram_tensor("x", [C, H_, W_], F32, kind="ExternalInput")
    nw = len(_W_ORDER)
    w_d = nc.dram_tensor("w_all", [128, nw * 128], BF16, kind="ExternalInput")
    raw_w = _build_weights()
    low_d = nc.dram_tensor("low", [C, H_, W_], BF16, kind="ExternalOutput")
    high_d = nc.dram_tensor("high", [C, H_, W_], BF16, kind="ExternalOutput")

    with tile.TileContext(nc) as tc:
        with (
            tc.tile_pool(name="consts", bufs=1) as consts,
            tc.tile_pool(name="xin", bufs=3) as xin,
            tc.tile_pool(name="sd", bufs=3) as sd,
            tc.tile_pool(name="mid", bufs=3) as mid,
            tc.tile_pool(name="hor", bufs=2) as hor,
            tc.tile_pool(name="lv1", bufs=2) as lv1,
            tc.tile_pool(name="outp", bufs=2) as outp,
            tc.tile_pool(name="pst1", bufs=1, space="PSUM") as pst1,
            tc.tile_pool(name="psL", bufs=1, space="PSUM") as psL,
            tc.tile_pool(name="psh", bufs=1, space="PSUM") as psh,
            tc.tile_pool(name="ps1", bufs=1, space="PSUM") as ps1,
        ):
            def load_weights():
                w_sb = consts.tile([128, nw, 128], BF16, tag="w_all")
                nc.sync.dma_start(
                    out=w_sb,
                    in_=w_d[:, :].rearrange("p (n w) -> p n w", w=128))
                wt = {}
                for i, name in enumerate(_W_ORDER):
                    rows = raw_w[name].shape[0]
                    wt[name] = w_sb[0:rows, i, :]
                return wt

            def blend255(qf, outf, name):
                """qf: [P, 4, 256] bf16 (col 255 = pad copy of 254).
                outf: [P, 4, 256]; out[i] = q[i] + w_i*(q[i-1]-q[i]),
                w_i = (i+0.5)/256; out[0] = q[0] (edge)."""
                nc.vector._custom_dve(
                    blendpg_op,
                    out=outf[:, :, 1:256],
                    in0=qf[:, :, 0:255],
                    in1=qf[:, :, 1:256],
                    s0=1.0 / 512.0,
                    s1=-255.0 / 256.0,
                )
                nc.scalar.copy(out=outf[:, :, 0:1], in_=qf[:, :, 0:1])

            def blend128(qf, outf):
                """qf: [P, 4, 128]; outf: [P, 4, 256] interleaved 2x
                upsample: out[2k]=.75q[k]+.25q[k-1], out[2k+1]=.75q[k]+.25q[k+1]."""
                nc.vector._custom_dve(
                    blend_op,
                    out=outf[:, :, 2:256:2],
                    in0=qf[:, :, 0:127],
                    in1=qf[:, :, 1:128],
                    s0=0.25,
                    s1=0.0,
                )
                nc.vector._custom_dve(
                    blend_op,
                    out=outf[:, :, 1:254:2],
                    in0=qf[:, :, 1:128],
                    in1=qf[:, :, 0:127],
                    s0=0.25,
                    s1=0.0,
                )
                nc.scalar.copy(out=outf[:, :, 0:1], in_=qf[:, :, 0:1])
                nc.scalar.copy(out=outf[:, :, 255:256],
                                      in_=qf[:, :, 127:128])

            # 2-iteration blocks: matmul/PSUM stages run per sub-iteration
            # (G=2 channels), elementwise stages run once per block (C4=4
            # channels) on doubled tiles to amortize per-op fixed costs.
            C4 = 2 * G
            n_blk = C // C4

            # software-pipelined loads: block k+1's loads are issued on the
            # sync queue BEFORE block k's stores, so the store's wait on the
            # block's final vector ops doesn't delay the next block's input
            pending = {}

            def issue_loads(blk):
                t = xin.tile([P, C4, 2, 256], F32, tag="xC")
                nc.sync.dma_start(
                    out=t,
                    in_=x_d[blk * C4:(blk + 1) * C4, :, :].rearrange(
                        "c (p r) w -> p c r w", r=2))
                pending[blk] = t

            # first input load beats the weight load onto the sync queue:
            # weights aren't needed until the first matmul
            issue_loads(0)
            wtile = load_weights()
            for blk in range(n_blk):
                c0b = blk * C4
                if blk + 1 < n_blk:
                    issue_loads(blk + 1)
                # block tiles: pages ordered (channel, row-parity)
                qLb = hor.tile([P, 2 * C4, 256], BF16, tag="qLb")
                qhb = hor.tile([P, 2 * C4, 256], BF16, tag="qhb")
                L0 = hor.tile([P, 2 * C4, 256], BF16, tag="L0")
                h0 = hor.tile([P, 2 * C4, 256], BF16, tag="h0")

                # ---- level-0 horizontal pair sum / |diff| (block-wide) -----
                xCb = pending.pop(blk)
                xCbf = xCb.rearrange("p c r w -> p (c r) w")   # [P, 8, 256]
                # 256-element page pitch keeps every page 4B-aligned
                # (255-pitch puts odd pages at a 2B offset -> 1x DVE mode)
                s_blk = sd.tile([P, 2 * C4, 256], BF16, tag="s")
                nc.vector.tensor_tensor(
                    out=s_blk[:, :, 0:255],
                    in0=xCbf[:, :, 0:255], in1=xCbf[:, :, 1:256],
                    op=Alu.add)
                ad_blk = sd.tile([P, 2 * C4, 256], BF16, tag="ad")
                nc.vector._custom_dve(
                    absdiff_op, out=ad_blk[:, :, 0:255],
                    in0=xCbf[:, :, 0:255], in1=xCbf[:, :, 1:256], s0=1.0)
                sv_b = s_blk.rearrange("p (c r) w -> p c r w", r=2)
                adv_b = ad_blk.rearrange("p (c r) w -> p c r w", r=2)

                for sub in (0, 1):
                    pg0 = sub * 2 * G           # first (c,r) page of this sub
                    cs = slice(sub * G, (sub + 1) * G)
                    sE = sv_b[:, cs, 0, 0:255]                # [P, G, 255]
                    sO = sv_b[:, cs, 1, 0:255]
                    adE = adv_b[:, cs, 0, 0:255]
                    adO = adv_b[:, cs, 1, 0:255]

                    # ---- t1 = row-diff of s, on the tensor engine ----------
                    t1EO = pst1.tile([P, 2, 512], F32, tag="t1EO")
                    nc.tensor.matmul(out=t1EO[:, 0, 0:510],
                                     lhsT=wtile["w_id"][:, :], rhs=sE,
                                     start=True, stop=False)
                    nc.tensor.matmul(out=t1EO[:, 0, 0:510],
                                     lhsT=wtile["w_idn"][:, :], rhs=sO,
                                     start=False, stop=True)
                    nc.tensor.matmul(out=t1EO[:, 1, 0:510],
                                     lhsT=wtile["w_id"][:, :], rhs=sO,
                                     start=True, stop=False)
                    nc.tensor.matmul(out=t1EO[:, 1, 0:510],
                                     lhsT=wtile["w_shn"][:, :], rhs=sE,
                                     start=False, stop=True)
                    a1 = mid.tile([P, 2, 510], BF16, tag=f"a1{sub}")
                    nc.scalar.activation(out=a1, in_=t1EO[:, :, 0:510],
                                         func=Act.Abs, scale=0.5)
                    a1E = a1[:, 0].rearrange("p (c w) -> p c w", w=255)
                    a1O = a1[:, 1].rearrange("p (c w) -> p c w", w=255)

                    # partition-shift |d| via the tensor engine into the t1
                    # buffer after the scalar abs has read it (no DMA latency
                    # on the mO chain); scalar evacuates to bf16 so the mO
                    # max runs in the 2x DVE mode
                    t1b = pst1.tile([P, 2, 512], F32, tag="t1EO")
                    nc.tensor.matmul(out=t1b[:, 0, 0:510],
                                     lhsT=wtile["w_shp"][:, :], rhs=adE,
                                     start=True, stop=True)
                    adE2t = mid.tile([127, G, 256], BF16,
                                     tag=f"adE2{sub}")
                    nc.scalar.copy(
                        out=adE2t[:, :, 0:255],
                        in_=t1b[0:127, 0, 0:510].rearrange(
                            "p (c w) -> p c w", w=255))
                    adE2 = adE2t[:, :, 0:255]

                    # ---- m = max(|d|,|d'|) ---------------------------------
                    # ch0 = a1 + m is folded into the Y_h matmul accumulation
                    mEt = mid.tile([P, G, 256], BF16, tag=f"mE{sub}")
                    mOt = mid.tile([127, G, 256], BF16, tag=f"mO{sub}")
                    mE = mEt[:, :, 0:255]
                    mO = mOt[:, :, 0:255]
                    nc.vector.tensor_tensor(out=mE, in0=adE, in1=adO,
                                            op=Alu.max)
                    nc.vector.tensor_tensor(out=mO, in0=adO[0:127], in1=adE2,
                                            op=Alu.max)

                    # ---- level-0 vertical matmuls (parity-split) -----------
                    Y_L = psL.tile([P, 2, 512], F32, tag="Y_L")
                    Y_h = psh.tile([P, 2, 512], F32, tag="Y_h")
                    for par in (0, 1):
                        nc.tensor.matmul(out=Y_L[:, par, 0:510],
                                         lhsT=wtile[f"w_va_{par}e"][:, :],
                                         rhs=sE, start=True, stop=False)
                        nc.tensor.matmul(out=Y_L[:, par, 0:510],
                                         lhsT=wtile[f"w_va_{par}o"][:, :],
                                         rhs=sO, start=False, stop=True)
                        nc.tensor.matmul(out=Y_h[:, par, 0:510],
                                         lhsT=wtile[f"w_vh_{par}e"][:, :],
                                         rhs=a1E, start=True, stop=False)
                        nc.tensor.matmul(out=Y_h[:, par, 0:510],
                                         lhsT=wtile[f"w_vh_{par}e"][:, :],
                                         rhs=mE, start=False, stop=False)
                        nc.tensor.matmul(out=Y_h[:, par, 0:510],
                                         lhsT=wtile[f"w_vh_{par}o"][:, :],
                                         rhs=a1O[0:127], start=False,
                                         stop=False)
                        nc.tensor.matmul(out=Y_h[:, par, 0:510],
                                         lhsT=wtile[f"w_vh_{par}o"][:, :],
                                         rhs=mO, start=False, stop=True)

                    # ---- evac to bf16 block-tile quarters + pad col --------
                    # (pad reads PSUM directly — independent of the main copy)
                    for q, Y in ((qLb, Y_L), (qhb, Y_h)):
                        qq = q[:, pg0:pg0 + 2 * G, :].rearrange(
                            "p (c r) w -> p c r w", r=2)
                        Yv = Y[:, :, 0:510].rearrange(
                            "p r (c w) -> p c r w", w=255)
                        nc.scalar.copy(out=qq[:, :, :, 0:255], in_=Yv)
                        nc.scalar.copy(out=qq[:, :, :, 255:256],
                                       in_=Yv[:, :, :, 254:255])

                # ---- level-0 horizontal resizes (block-wide) ---------------
                blend255(qLb, L0, "L0")
                blend255(qhb, h0, "h0")

                # ---- level-1 elementwise (block-wide) ----------------------
                s2 = lv1.tile([P, 2 * C4, 128], BF16, tag="s2")
                nc.vector.tensor_tensor(out=s2, in0=L0[:, :, 0:256:2],
                                        in1=L0[:, :, 1:256:2], op=Alu.add)
                ad2 = lv1.tile([P, 2 * C4, 128], BF16, tag="ad2")
                nc.vector._custom_dve(
                    absdiff_op, out=ad2,
                    in0=L0[:, :, 0:256:2], in1=L0[:, :, 1:256:2], s0=1.0)
                s2v = s2.rearrange("p (c r) w -> p c r w", r=2)
                ad2v = ad2.rearrange("p (c r) w -> p c r w", r=2)
                t1c = lv1.tile([P, C4, 128], BF16, tag="t1c")
                nc.vector.tensor_tensor(out=t1c, in0=s2v[:, :, 0],
                                        in1=s2v[:, :, 1], op=Alu.subtract)
                a1b = lv1.tile([P, C4, 128], BF16, tag="a1b")
                nc.scalar.activation(out=a1b, in_=t1c, func=Act.Abs, scale=0.5)
                m1 = lv1.tile([P, C4, 128], BF16, tag="m1")
                nc.vector.tensor_tensor(out=m1, in0=ad2v[:, :, 0],
                                        in1=ad2v[:, :, 1], op=Alu.max)

                # ---- level-1 vertical matmuls + evac (per sub, PSUM) -------
                # lsum1 = s2e+s2o and ch1 = a1b+m1 are folded into the
                # matmul accumulations (weights reused -> no extra LDWEIGHTS)
                # qq block tile: pages 0..7 = low (c,r), 8..15 = h1 (c,r)
                qq = lv1.tile([P, 4 * C4, 128], BF16, tag="qq")
                for sub in (0, 1):
                    Y_lo = ps1.tile([P, 2, G, 128], F32, tag="Y_lo")
                    Y_h1 = ps1.tile([P, 2, G, 128], F32, tag="Y_h1")
                    sl = slice(sub * G, (sub + 1) * G)
                    for par in (0, 1):
                        nc.tensor.matmul(out=Y_lo[:, par],
                                         lhsT=wtile[f"w_vq_{par}"][:, :],
                                         rhs=s2v[:, sl, 0], start=True,
                                         stop=False)
                        nc.tensor.matmul(out=Y_lo[:, par],
                                         lhsT=wtile[f"w_vq_{par}"][:, :],
                                         rhs=s2v[:, sl, 1], start=False,
                                         stop=True)
                        nc.tensor.matmul(out=Y_h1[:, par],
                                         lhsT=wtile[f"w_vq_{par}"][:, :],
                                         rhs=a1b[:, sl], start=True,
                                         stop=False)
                        nc.tensor.matmul(out=Y_h1[:, par],
                                         lhsT=wtile[f"w_vq_{par}"][:, :],
                                         rhs=m1[:, sl], start=False,
                                         stop=True)
                    nc.scalar.copy(
                        out=qq[:, sub * 2 * G:(sub + 1) * 2 * G, :].rearrange(
                            "p (c r) w -> p c r w", r=2),
                        in_=Y_lo.rearrange("p r c w -> p c r w"))
                    nc.scalar.copy(
                        out=qq[:, 2 * C4 + sub * 2 * G:
                               2 * C4 + (sub + 1) * 2 * G, :].rearrange(
                            "p (c r) w -> p c r w", r=2),
                        in_=Y_h1.rearrange("p r c w -> p c r w"))

                # ---- level-1 horizontal (2x upsample, low+h1 together) -----
                lowhi = outp.tile([P, 4 * C4, 256], BF16, tag="lowhi")
                blend128(qq, lowhi)
                # high = h0 + h1 via identity-matmul accumulation (tensor has
                # slack; scalar evacuates) — two 2-bank halves through the
                # freed Y_h PSUM buffer
                highI = outp.tile([P, 2 * C4, 256], BF16, tag="highI")
                h1I = lowhi[:, 2 * C4:4 * C4, :]
                if blk == n_blk - 1:
                    # last block: direct vector add — skips the tensor+scalar
                    # round trip that would otherwise gate the final stores
                    nc.vector.tensor_tensor(out=highI, in0=h0, in1=h1I,
                                            op=Alu.add)
                else:
                    for half in (0, 1):
                        ph = psh.tile([P, 2, 512], F32, tag="Y_h")
                        hs = slice(half * C4, (half + 1) * C4)
                        for q in (0, 1):
                            qs = slice(half * C4 + q * 2,
                                       half * C4 + q * 2 + 2)
                            nc.tensor.matmul(
                                out=ph[:, q, 0:512],
                                lhsT=wtile["w_id"][:, :],
                                rhs=h0[:, qs, :], start=True, stop=False)
                            nc.tensor.matmul(
                                out=ph[:, q, 0:512],
                                lhsT=wtile["w_id"][:, :],
                                rhs=h1I[:, qs, :], start=False, stop=True)
                        nc.scalar.copy(
                            out=highI[:, hs, :],
                            in_=ph.rearrange("p a w -> p (a w)")[:, 0:1024]
                            .rearrange("p (c w) -> p c w", w=256))

                # ---- store (bf16, row-pair chunks) -------------------------
                nc.sync.dma_start(
                    out=low_d[c0b:c0b + C4, :, :].rearrange(
                        "c (p r) w -> p c r w", r=2),
                    in_=lowhi[:, 0:2 * C4, :].rearrange(
                        "p (c r) w -> p c r w", r=2))
                nc.sync.dma_start(
                    out=high_d[c0b:c0b + C4, :, :].rearrange(
                        "c (p r) w -> p c r w", r=2),
                    in_=highI.rearrange("p (c r) w -> p c r w", r=2))

    nc.compile()
    _NC_CACHE[C] = nc
    return nc


# ----------------------------------------------------------------------------
# host entry points
# ----------------------------------------------------------------------------

_RUNNER = None


def _get_runner():
    """Builds (once) a cached sharded jit executable over the 8 cores."""
    global _RUNNER
    if _RUNNER is not None:
        return _RUNNER

    import jax
    from jax.sharding import Mesh, PartitionSpec, NamedSharding
    from jax.experimental.shard_map import shard_map
    import concourse.mybir as mybir
    from concourse import bass2jax
    from concourse.bass2jax import _bass_exec_p, partition_id_tensor

    bass2jax.install_neuronx_cc_hook()
    nc = build_nc(C_)

    partition_name = nc.partition_id_tensor.name if nc.partition_id_tensor else None
    in_names, out_names, out_avals = [], [], []
    for alloc in nc.m.functions[0].allocations:
        if not isinstance(alloc, mybir.MemoryLocationSet):
            continue
        name = alloc.memorylocations[0].name
        if alloc.kind == "ExternalInput":
            if name != partition_name:
                in_names.append(name)
        elif alloc.kind == "ExternalOutput":
            out_names.append(name)
            out_avals.append(jax.core.ShapedArray(
                tuple(alloc.tensor_shape), mybir.dt.np(alloc.dtype)))
    n_params = len(in_names)
    all_in_names = list(in_names) + list(out_names)
    if partition_name is not None:
        all_in_names.append(partition_name)

    def _body(*args):
        operands = list(args)
        if partition_name is not None:
            operands.append(partition_id_tensor())
        return tuple(_bass_exec_p.bind(
            *operands,
            out_avals=tuple(out_avals),
            in_names=tuple(all_in_names),
            out_names=tuple(out_names),
            lowering_input_output_aliases=(),
            sim_require_finite=True,
            sim_require_nnan=True,
            nc=nc,
        ))

    devices = jax.devices()[:NCORES]
    mesh = Mesh(np.asarray(devices), ("core",))
    n_in = n_params + len(out_names)
    sharded = jax.jit(shard_map(
        _body, mesh=mesh,
        in_specs=(PartitionSpec("core"),) * n_in,
        out_specs=(PartitionSpec("core"),) * len(out_names),
        check_rep=False))

    shard0 = NamedSharding(mesh, PartitionSpec("core"))
    wt = _weights()
    static = {}
    for name in in_names:
        if name == "x":
            continue
        arr = np.concatenate([wt[name]] * NCORES, axis=0)
        static[name] = jax.device_put(arr, shard0)
    for name, aval in zip(out_names, out_avals):
        z = np.zeros((aval.shape[0] * NCORES,) + tuple(aval.shape[1:]),
                     dtype=aval.dtype)
        static[name] = jax.device_put(z, shard0)

    def run(x_global):
        ops = []
        for name in in_names:
            ops.append(x_global if name == "x" else static[name])
        for name in out_names:
            ops.append(static[name])
        outs = sharded(*ops)
        return dict(zip(out_names, outs))

    _RUNNER = (run, shard0)
    return _RUNNER


def _run_device(x, trace=False):
    """x: [8, 64, 256, 256] fp32. Returns (low, high, results_obj)."""
    if trace:
        import shutil
        from concourse import bass_utils
        nc = build_nc(C_)
        wt = _weights()
        in_maps = [dict(wt, x=np.ascontiguousarray(x[b])) for b in range(NCORES)]
        shutil.rmtree("/tmp/bass_trace", ignore_errors=True)
        import os
        os.makedirs("/tmp/bass_trace", exist_ok=True)
        res = bass_utils.run_bass_kernel_spmd(
            nc, in_maps, core_ids=list(range(NCORES)), trace=True,
            tmpdir="/tmp/bass_trace")
        low = np.stack([np.asarray(res.results[b]["low"]) for b in range(NCORES)])
        high = np.stack([np.asarray(res.results[b]["high"]) for b in range(NCORES)])
        return low.astype(np.float32), high.astype(np.float32), res

    run, _ = _get_runner()
    outs = run(np.ascontiguousarray(x).reshape(B_ * C_, H_, W_))
    low = np.asarray(outs["low"]).reshape(B_, C_, H_, W_).astype(np.float32)
    high = np.asarray(outs["high"]).reshape(B_, C_, H_, W_).astype(np.float32)
    return low, high, None


def _fallback(x, level):
    """Numpy port of the reference for unexpected shapes/levels."""
    xl = x.astype(np.float64)
    Bb, Cc, H, W = xl.shape
    low = xl
    high = np.zeros_like(xl)

    def up(a, n_r, n_c):
        Mr = _resize_matrix(a.shape[-2], n_r)
        Mc = _resize_matrix(a.shape[-1], n_c)
        return np.einsum("ij,...jk,lk->...il", Mr, a, Mc)

    for lv in range(level):
        stride = 2 ** lv
        if H // stride < 2 or W // stride < 2:
            break
        x00 = low[..., 0:H - 1:stride, 0:W - 1:stride]
        x01 = low[..., 0:H - 1:stride, 1:W:stride]
        x10 = low[..., 1:H:stride, 0:W - 1:stride]
        x11 = low[..., 1:H:stride, 1:W:stride]
        ll = (x00 + x01 + x10 + x11) * 0.25
        lh = (x00 + x01 - x10 - x11) * 0.25
        hl = (x00 - x01 + x10 - x11) * 0.25
        hh = (x00 - x01 - x10 + x11) * 0.25
        ch = np.abs(lh) + np.abs(hl) + np.abs(hh)
        high = high + up(ch, H, W)
        low = up(ll, H, W)
    if level > 0:
        high = high / level
    return low.astype(np.float32), high.astype(np.float32)


def kernel(x, level):
    x = np.asarray(x, dtype=np.float32)
    level = int(level)
    if level != 2 or x.shape != (B_, C_, H_, W_):
        return _fallback(x, level)
    low, high, _ = _run_device(x)
    return low, high


# revision 60
# speedup vs baseline: 1.0399x; 1.0319x over previous
"""HaarWavelet2D (level=2) Trainium2 kernel — fused-DVE redesign.

Contract: kernel(x, level) with x [8, 64, 256, 256] fp32, level=2.
Returns (low_freq, high_freq), each [8, 64, 256, 256] fp32 — matching the
jax reference (2-level Haar decomposition with bilinear resizes).

Sharding: data-parallel over batch — core b processes x[b] (64 channels),
in 16 blocks of 4 channels (2 sub-iterations of G=2 for the PSUM-bound
matmul stages, block-wide elementwise stages to amortize per-op fixed
costs). Layout: row-pairs in partitions (partition p holds rows 2p/2p+1
as a parity dim), (channel, parity, col) in the free dims, so every HBM
transfer moves 2KB-contiguous chunks; loads are software-pipelined one
block ahead of the stores on the sync queue.

Engine placement:
  DVE    : s = pair-sum / |d| (fused absdiff, straight from f32),
           max/ch0/ch1 assembly, fused Idx/PageIdx-blend horizontal
           resizes (255->256 in one paged call, 128->256 interleaved)
  Tensor : row-diff t1 via +/-identity matmuls, all vertical resizes
           (parity-split banded matrices, bf16, one packed weight DMA)
  Scalar : |0.5*t1| from PSUM, PSUM->SBUF bf16 evacuations + pad cols
  DMA    : 1 load (f32) + 1 partition-shift copy per sub, 2 bf16
           stores per block
Outputs are written bf16 to HBM and upcast on the host (halves store
traffic; validated rel err ~4e-3 vs the 2e-2 gate in model.py).
"""

import sys

if "/opt/trn_rl_repo" not in sys.path:
    sys.path.insert(0, "/opt/trn_rl_repo")

import numpy as np
import ml_dtypes

BF = ml_dtypes.bfloat16

B_, C_, H_, W_ = 8, 64, 256, 256
NCORES = 8
G = 2  # channels per inner iteration


# ----------------------------------------------------------------------------
# custom fused DVE ops (registered into dve_ops.OPS at import)
# ----------------------------------------------------------------------------

_DVE_OPS = None


def _register_dve_ops():
    global _DVE_OPS
    if _DVE_OPS is not None:
        return _DVE_OPS
    from concourse.dve_spec import (
        Spec, Src0, Src1, C0, C1, Idx, Zero, One, PageIdx, maxx, scan, lower,
        AluOp,
    )
    from concourse import dve_ops as DO
    from concourse.dve_uop import DveOpSpec

    def mk(name, spec, subdim=False):
        for op in DO.OPS:
            if op.name == name:
                return op
        shas = {}
        for ver in ("v3", "v4"):
            try:
                uops = lower(spec, ver=ver)
                shas[ver] = DveOpSpec(
                    name=name, opcode=0, uops=uops, rd1_en=True
                ).sha(ver)
            except Exception:
                pass
        op = DO.DveOp(name, spec, subdim=subdim, uops_sha=shas)
        DO.OPS.append(op)
        DO.CUSTOM_DVE_SPECS[op.name] = op.spec
        DO._SUB_OPCODE_FOR_NAME[op.name] = (
            DO._CUSTOM_DVE_ROW_BASE + len(DO.OPS) - 1)
        assert DO._SUB_OPCODE_FOR_NAME[op.name] < 0x20
        return op

    # out = |in0 - in1| * s0
    absdiff = mk(
        "HW19_ABSDIFF",
        Spec(body=maxx(Src0 - Src1, Src1 - Src0) * C0,
             reference=lambda in0, in1, s0, s1, imm2: np.abs(in0 - in1) * s0))
    # out = in1 + (s0 + Idx*s1) * (in0 - in1)   (linear-in-index blend)
    blend = mk(
        "HW19_BLEND",
        Spec(body=Src1 + (C0 + Idx * C1) * (Src0 - Src1),
             reference=lambda in0, in1, s0, s1, imm2: in1 + (
                 s0 + np.arange(in0.shape[-1]) * s1) * (in0 - in1)))
    # paged 255->256 resize: w = (k_in_page + 1.5)/256 with s0 = 1/512 and
    # s1 = -255/256 cancelling the global-Idx page offset
    def _blendpg_ref(in0, in1, s0, s1, imm2):
        S, N = in0.shape[-2], in0.shape[-1]
        k = np.arange(S * N).reshape(S, N)
        w = s0 + (k + 1) * 2 * s0 + np.arange(S)[:, None] * s1
        return in1 + w * (in0 - in1)
    blendpg = mk(
        "HW19_BLENDPG",
        Spec(body=Src1 + (scan(AluOp.ADD, C0 + C0, init=C0)
                          + PageIdx(Zero, C1)) * (Src0 - Src1),
             reference=_blendpg_ref),
        subdim=True)
    # paged blend with independent slope/offset: w = s0*k_in_page + (s1+s0)
    # (page correction uses the hardware One: pages are 1/s0 elements wide)
    def _blendpg2_ref(in0, in1, s0, s1, imm2):
        S, N = in0.shape[-2], in0.shape[-1]
        k = np.arange(S * N).reshape(S, N)
        w = s1 + (k + 1) * s0 - np.arange(S)[:, None]
        return in1 + w * (in0 - in1)
    blendpg2 = mk(
        "HW19_BLENDPG2",
        Spec(body=Src1 + (scan(AluOp.ADD, C0, init=C1)
                          - PageIdx(Zero, One)) * (Src0 - Src1),
             reference=_blendpg2_ref),
        subdim=True)
    _DVE_OPS = (absdiff, blend, blendpg, blendpg2)
    return _DVE_OPS


# ----------------------------------------------------------------------------
# host-side weight construction
# ----------------------------------------------------------------------------

def _resize_matrix(n, N):
    M = np.zeros((N, n), dtype=np.float64)
    for i in range(N):
        c = (i + 0.5) * n / N - 0.5
        j0 = int(np.floor(c))
        f = c - j0
        M[i, min(max(j0, 0), n - 1)] += 1.0 - f
        M[i, min(max(j0 + 1, 0), n - 1)] += f
    return M


def _build_weights():
    V255 = _resize_matrix(255, 256)
    V128 = _resize_matrix(128, 256)
    Sv1 = np.zeros((255, 256))
    for r in range(255):
        Sv1[r, r] = 1.0
        Sv1[r, r + 1] = 1.0
    Va = 0.25 * (V255 @ Sv1)      # [256, 256] vertical resize+pair-sum fold
    V255s = 0.25 * V255           # [256, 255]
    V128q = 0.25 * V128           # [256, 128]

    w = {}
    for par in (0, 1):
        # lhsT convention: out[po,f] = sum_pi lhsT[pi,po] * rhs[pi,f]
        w[f"w_va_{par}e"] = Va[par::2, 0::2].T        # [128,128]
        w[f"w_va_{par}o"] = Va[par::2, 1::2].T
        w[f"w_vh_{par}e"] = V255s[par::2, 0::2].T     # [128,128]
        w[f"w_vh_{par}o"] = V255s[par::2, 1::2].T     # [127,128]
        w[f"w_vq_{par}"] = V128q[par::2, :].T         # [128,128]
    w["w_id"] = np.eye(128)
    w["w_idn"] = -np.eye(128)
    # t1O[po] = sO[po] - sE[po+1]:  lhsT[pi,po] = -1 iff pi == po+1
    w["w_shn"] = np.diag(-np.ones(127), k=-1)
    # adE2[po] = adE[po+1]: positive partition shift
    w["w_shp"] = np.diag(np.ones(127), k=-1)
    return {k: v.astype(BF) for k, v in w.items()}


# fixed packing order for the single-DMA weight tile; [127,128] weights are
# zero-row-padded (sliced back to 127 partitions at the call site)
_W_ORDER = ["w_va_0e", "w_va_0o", "w_va_1e", "w_va_1o",
            "w_vh_0e", "w_vh_0o", "w_vh_1e", "w_vh_1o",
            "w_vq_0", "w_vq_1", "w_id", "w_idn", "w_shn", "w_shp"]


def _pack_weights(w):
    cols = []
    for k in _W_ORDER:
        a = w[k]
        if a.shape[0] < 128:
            a = np.concatenate(
                [a, np.zeros((128 - a.shape[0], a.shape[1]), dtype=a.dtype)])
        cols.append(a)
    return np.stack(cols, axis=1).reshape(128, -1)  # [128, 13*128]


_WEIGHTS = None


def _weights():
    """Device-side operand dict: one packed weight tensor."""
    global _WEIGHTS
    if _WEIGHTS is None:
        _WEIGHTS = {"w_all": _pack_weights(_build_weights())}
    return _WEIGHTS


# ----------------------------------------------------------------------------
# bass program
# ----------------------------------------------------------------------------

_NC_CACHE = {}


def build_nc(C=C_):
    if C in _NC_CACHE:
        return _NC_CACHE[C]

    import concourse.bass as bass  # noqa: F401
    import concourse.bacc as bacc
    import concourse.tile as tile
    import concourse.mybir as mybir

    absdiff_op, blend_op, blendpg_op, blendpg2_op = _register_dve_ops()

    F32 = mybir.dt.float32
    BF16 = mybir.dt.bfloat16
    Alu = mybir.AluOpType
    Act = mybir.ActivationFunctionType
    P = 128

    nc = bacc.Bacc("TRN2", target_bir_lowering=False)
    x_d = nc.dram_tensor("x", [C, H_, W_], F32, kind="ExternalInput")
    nw = len(_W_ORDER)
    w_d = nc.dram_tensor("w_all", [128, nw * 128], BF16, kind="ExternalInput")
    raw_w = _build_weights()
    low_d = nc.dram_tensor("low", [C, H_, W_], BF16, kind="ExternalOutput")
    high_d = nc.dram_tensor("high", [C, H_, W_], BF16, kind="ExternalOutput")

    with tile.TileContext(nc) as tc:
        with (
            tc.tile_pool(name="consts", bufs=1) as consts,
            tc.tile_pool(name="xin", bufs=3) as xin,
            tc.tile_pool(name="sd", bufs=3) as sd,
            tc.tile_pool(name="mid", bufs=3) as mid,
            tc.tile_pool(name="hor", bufs=2) as hor,
            tc.tile_pool(name="lv1", bufs=2) as lv1,
            tc.tile_pool(name="outp", bufs=2) as outp,
            tc.tile_pool(name="pst1", bufs=1, space="PSUM") as pst1,
            tc.tile_pool(name="psL", bufs=1, space="PSUM") as psL,
            tc.tile_pool(name="psh", bufs=1, space="PSUM") as psh,
            tc.tile_pool(name="ps1", bufs=1, space="PSUM") as ps1,
        ):
            def load_weights():
                w_sb = consts.tile([128, nw, 128], BF16, tag="w_all")
                nc.sync.dma_start(
                    out=w_sb,
                    in_=w_d[:, :].rearrange("p (n w) -> p n w", w=128))
                wt = {}
                for i, name in enumerate(_W_ORDER):
                    rows = raw_w[name].shape[0]
                    wt[name] = w_sb[0:rows, i, :]
                return wt

            def blend255(qf, outf, name):
                """qf: [P, 4, 256] bf16 (col 255 = pad copy of 254).
                outf: [P, 4, 256]; out[i] = q[i] + w_i*(q[i-1]-q[i]),
                w_i = (i+0.5)/256; out[0] = q[0] (edge)."""
                nc.vector._custom_dve(
                    blendpg_op,
                    out=outf[:, :, 1:256],
                    in0=qf[:, :, 0:255],
                    in1=qf[:, :, 1:256],
                    s0=1.0 / 512.0,
                    s1=-255.0 / 256.0,
                )
                nc.scalar.copy(out=outf[:, :, 0:1], in_=qf[:, :, 0:1])

            def blend128(qf, outf):
                """qf: [P, 4, 128]; outf: [P, 4, 256] interleaved 2x
                upsample: out[2k]=.75q[k]+.25q[k-1], out[2k+1]=.75q[k]+.25q[k+1]."""
                nc.vector._custom_dve(
                    blend_op,
                    out=outf[:, :, 2:256:2],
                    in0=qf[:, :, 0:127],
                    in1=qf[:, :, 1:128],
                    s0=0.25,
                    s1=0.0,
                )
                nc.vector._custom_dve(
                    blend_op,
                    out=outf[:, :, 1:254:2],
                    in0=qf[:, :, 1:128],
                    in1=qf[:, :, 0:127],
                    s0=0.25,
                    s1=0.0,
                )
                nc.scalar.copy(out=outf[:, :, 0:1], in_=qf[:, :, 0:1])
                nc.scalar.copy(out=outf[:, :, 255:256],
                                      in_=qf[:, :, 127:128])

            # 2-iteration blocks: matmul/PSUM stages run per sub-iteration
            # (G=2 channels), elementwise stages run once per block (C4=4
            # channels) on doubled tiles to amortize per-op fixed costs.
            C4 = 2 * G
            n_blk = C // C4

            # software-pipelined loads: block k+1's loads are issued on the
            # sync queue BEFORE block k's stores, so the store's wait on the
            # block's final vector ops doesn't delay the next block's input
            pending = {}

            def issue_loads(blk):
                t = xin.tile([P, C4, 2, 256], F32, tag="xC")
                nc.sync.dma_start(
                    out=t,
                    in_=x_d[blk * C4:(blk + 1) * C4, :, :].rearrange(
                        "c (p r) w -> p c r w", r=2))
                pending[blk] = t

            # first input load beats the weight load onto the sync queue:
            # weights aren't needed until the first matmul
            issue_loads(0)
            wtile = load_weights()
            for blk in range(n_blk):
                c0b = blk * C4
                if blk + 1 < n_blk:
                    issue_loads(blk + 1)
                # block tiles: pages ordered (channel, row-parity)
                qLb = hor.tile([P, 2 * C4, 256], BF16, tag="qLb")
                qhb = hor.tile([P, 2 * C4, 256], BF16, tag="qhb")
                L0 = hor.tile([P, 2 * C4, 256], BF16, tag="L0")
                h0 = hor.tile([P, 2 * C4, 256], BF16, tag="h0")

                # ---- level-0 horizontal pair sum / |diff| (block-wide) -----
                xCb = pending.pop(blk)
                xCbf = xCb.rearrange("p c r w -> p (c r) w")   # [P, 8, 256]
                # 256-element page pitch keeps every page 4B-aligned
                # (255-pitch puts odd pages at a 2B offset -> 1x DVE mode)
                s_blk = sd.tile([P, 2 * C4, 256], BF16, tag="s")
                nc.vector.tensor_tensor(
                    out=s_blk[:, :, 0:255],
                    in0=xCbf[:, :, 0:255], in1=xCbf[:, :, 1:256],
                    op=Alu.add)
                ad_blk = sd.tile([P, 2 * C4, 256], BF16, tag="ad")
                nc.vector._custom_dve(
                    absdiff_op, out=ad_blk[:, :, 0:255],
                    in0=xCbf[:, :, 0:255], in1=xCbf[:, :, 1:256], s0=1.0)
                sv_b = s_blk.rearrange("p (c r) w -> p c r w", r=2)
                adv_b = ad_blk.rearrange("p (c r) w -> p c r w", r=2)

                for sub in (0, 1):
                    pg0 = sub * 2 * G           # first (c,r) page of this sub
                    cs = slice(sub * G, (sub + 1) * G)
                    sE = sv_b[:, cs, 0, 0:255]                # [P, G, 255]
                    sO = sv_b[:, cs, 1, 0:255]
                    adE = adv_b[:, cs, 0, 0:255]
                    adO = adv_b[:, cs, 1, 0:255]

                    # ---- t1 = row-diff of s, on the tensor engine ----------
                    t1EO = pst1.tile([P, 2, 512], F32, tag="t1EO")
                    nc.tensor.matmul(out=t1EO[:, 0, 0:510],
                                     lhsT=wtile["w_id"][:, :], rhs=sE,
                                     start=True, stop=False)
                    nc.tensor.matmul(out=t1EO[:, 0, 0:510],
                                     lhsT=wtile["w_idn"][:, :], rhs=sO,
                                     start=False, stop=True)
                    nc.tensor.matmul(out=t1EO[:, 1, 0:510],
                                     lhsT=wtile["w_id"][:, :], rhs=sO,
                                     start=True, stop=False)
                    nc.tensor.matmul(out=t1EO[:, 1, 0:510],
                                     lhsT=wtile["w_shn"][:, :], rhs=sE,
                                     start=False, stop=True)
                    a1 = mid.tile([P, 2, 510], BF16, tag=f"a1{sub}")
                    nc.scalar.activation(out=a1, in_=t1EO[:, :, 0:510],
                                         func=Act.Abs, scale=0.5)
                    a1E = a1[:, 0].rearrange("p (c w) -> p c w", w=255)
                    a1O = a1[:, 1].rearrange("p (c w) -> p c w", w=255)

                    # partition-shift |d| via the tensor engine into the t1
                    # buffer after the scalar abs has read it (no DMA latency
                    # on the mO chain); scalar evacuates to bf16 so the mO
                    # max runs in the 2x DVE mode
                    t1b = pst1.tile([P, 2, 512], F32, tag="t1EO")
                    nc.tensor.matmul(out=t1b[:, 0, 0:510],
                                     lhsT=wtile["w_shp"][:, :], rhs=adE,
                                     start=True, stop=True)
                    adE2t = mid.tile([127, G, 256], BF16,
                                     tag=f"adE2{sub}")
                    nc.scalar.copy(
                        out=adE2t[:, :, 0:255],
                        in_=t1b[0:127, 0, 0:510].rearrange(
                            "p (c w) -> p c w", w=255))
                    adE2 = adE2t[:, :, 0:255]

                    # ---- m = max(|d|,|d'|) ---------------------------------
                    # ch0 = a1 + m is folded into the Y_h matmul accumulation
                    mEt = mid.tile([P, G, 256], BF16, tag=f"mE{sub}")
                    mOt = mid.tile([127, G, 256], BF16, tag=f"mO{sub}")
                    mE = mEt[:, :, 0:255]
                    mO = mOt[:, :, 0:255]
                    nc.vector.tensor_tensor(out=mE, in0=adE, in1=adO,
                                            op=Alu.max)
                    nc.vector.tensor_tensor(out=mO, in0=adO[0:127], in1=adE2,
                                            op=Alu.max)

                    # ---- level-0 vertical matmuls (parity-split) -----------
                    Y_L = psL.tile([P, 2, 512], F32, tag="Y_L")
                    Y_h = psh.tile([P, 2, 512], F32, tag="Y_h")
                    for par in (0, 1):
                        nc.tensor.matmul(out=Y_L[:, par, 0:510],
                                         lhsT=wtile[f"w_va_{par}e"][:, :],
                                         rhs=sE, start=True, stop=False)
                        nc.tensor.matmul(out=Y_L[:, par, 0:510],
                                         lhsT=wtile[f"w_va_{par}o"][:, :],
                                         rhs=sO, start=False, stop=True)
                        nc.tensor.matmul(out=Y_h[:, par, 0:510],
                                         lhsT=wtile[f"w_vh_{par}e"][:, :],
                                         rhs=a1E, start=True, stop=False)
                        nc.tensor.matmul(out=Y_h[:, par, 0:510],
                                         lhsT=wtile[f"w_vh_{par}e"][:, :],
                                         rhs=mE, start=False, stop=False)
                        nc.tensor.matmul(out=Y_h[:, par, 0:510],
                                         lhsT=wtile[f"w_vh_{par}o"][:, :],
                                         rhs=a1O[0:127], start=False,
                                         stop=False)
                        nc.tensor.matmul(out=Y_h[:, par, 0:510],
                                         lhsT=wtile[f"w_vh_{par}o"][:, :],
                                         rhs=mO, start=False, stop=True)

                    # ---- evac to bf16 block-tile quarters + pad col --------
                    # (pad reads PSUM directly — independent of the main copy)
                    for q, Y in ((qLb, Y_L), (qhb, Y_h)):
                        qq = q[:, pg0:pg0 + 2 * G, :].rearrange(
                            "p (c r) w -> p c r w", r=2)
                        Yv = Y[:, :, 0:510].rearrange(
                            "p r (c w) -> p c r w", w=255)
                        nc.scalar.copy(out=qq[:, :, :, 0:255], in_=Yv)
                        nc.scalar.copy(out=qq[:, :, :, 255:256],
                                       in_=Yv[:, :, :, 254:255])

                # ---- level-0 horizontal resizes (block-wide) ---------------
                blend255(qLb, L0, "L0")
                blend255(qhb, h0, "h0")

                # ---- level-1 elementwise (block-wide) ----------------------
                s2 = lv1.tile([P, 2 * C4, 128], BF16, tag="s2")
                nc.vector.tensor_tensor(out=s2, in0=L0[:, :, 0:256:2],
                                        in1=L0[:, :, 1:256:2], op=Alu.add)
                ad2 = lv1.tile([P, 2 * C4, 128], BF16, tag="ad2")
                nc.vector._custom_dve(
                    absdiff_op, out=ad2,
                    in0=L0[:, :, 0:256:2], in1=L0[:, :, 1:256:2], s0=1.0)
                s2v = s2.rearrange("p (c r) w -> p c r w", r=2)
                ad2v = ad2.rearrange("p (c r) w -> p c r w", r=2)
                # t1c = s2e - s2o on the tensor engine (borrows the Y_lo
                # PSUM buffer before its real use this block); scalar abs
                # reads PSUM directly
                t1p = ps1.tile([P, 2, G, 128], F32, tag="Y_lo")
                t1pf = t1p.rearrange("p a g w -> p (a g w)")
                nc.tensor.matmul(out=t1pf[:, 0:512],
                                 lhsT=wtile["w_id"][:, :],
                                 rhs=s2v[:, :, 0], start=True, stop=False)
                nc.tensor.matmul(out=t1pf[:, 0:512],
                                 lhsT=wtile["w_idn"][:, :],
                                 rhs=s2v[:, :, 1], start=False, stop=True)
                a1b = lv1.tile([P, C4, 128], BF16, tag="a1b")
                nc.scalar.activation(
                    out=a1b,
                    in_=t1pf[:, 0:512].rearrange("p (c w) -> p c w", w=128),
                    func=Act.Abs, scale=0.5)
                m1 = lv1.tile([P, C4, 128], BF16, tag="m1")
                nc.vector.tensor_tensor(out=m1, in0=ad2v[:, :, 0],
                                        in1=ad2v[:, :, 1], op=Alu.max)

                # ---- level-1 vertical matmuls + evac (per sub, PSUM) -------
                # lsum1 = s2e+s2o and ch1 = a1b+m1 are folded into the
                # matmul accumulations (weights reused -> no extra LDWEIGHTS)
                # qq block tile: pages 0..7 = low (c,r), 8..15 = h1 (c,r)
                qq = lv1.tile([P, 4 * C4, 128], BF16, tag="qq")
                for sub in (0, 1):
                    Y_lo = ps1.tile([P, 2, G, 128], F32, tag="Y_lo")
                    Y_h1 = ps1.tile([P, 2, G, 128], F32, tag="Y_h1")
                    sl = slice(sub * G, (sub + 1) * G)
                    for par in (0, 1):
                        nc.tensor.matmul(out=Y_lo[:, par],
                                         lhsT=wtile[f"w_vq_{par}"][:, :],
                                         rhs=s2v[:, sl, 0], start=True,
                                         stop=False)
                        nc.tensor.matmul(out=Y_lo[:, par],
                                         lhsT=wtile[f"w_vq_{par}"][:, :],
                                         rhs=s2v[:, sl, 1], start=False,
                                         stop=True)
                        nc.tensor.matmul(out=Y_h1[:, par],
                                         lhsT=wtile[f"w_vq_{par}"][:, :],
                                         rhs=a1b[:, sl], start=True,
                                         stop=False)
                        nc.tensor.matmul(out=Y_h1[:, par],
                                         lhsT=wtile[f"w_vq_{par}"][:, :],
                                         rhs=m1[:, sl], start=False,
                                         stop=True)
                    nc.scalar.copy(
                        out=qq[:, sub * 2 * G:(sub + 1) * 2 * G, :].rearrange(
                            "p (c r) w -> p c r w", r=2),
                        in_=Y_lo.rearrange("p r c w -> p c r w"))
                    nc.scalar.copy(
                        out=qq[:, 2 * C4 + sub * 2 * G:
                               2 * C4 + (sub + 1) * 2 * G, :].rearrange(
                            "p (c r) w -> p c r w", r=2),
                        in_=Y_h1.rearrange("p r c w -> p c r w"))

                # ---- level-1 horizontal (2x upsample, low+h1 together) -----
                lowhi = outp.tile([P, 4 * C4, 256], BF16, tag="lowhi")
                blend128(qq, lowhi)
                # high = h0 + h1 via identity-matmul accumulation (tensor has
                # slack; scalar evacuates) — two 2-bank halves through the
                # freed Y_h PSUM buffer
                highI = outp.tile([P, 2 * C4, 256], BF16, tag="highI")
                h1I = lowhi[:, 2 * C4:4 * C4, :]
                if blk == n_blk - 1:
                    # last block: direct vector add — skips the tensor+scalar
                    # round trip that would otherwise gate the final stores
                    nc.vector.tensor_tensor(out=highI, in0=h0, in1=h1I,
                                            op=Alu.add)
                else:
                    for half in (0, 1):
                        ph = psh.tile([P, 2, 512], F32, tag="Y_h")
                        hs = slice(half * C4, (half + 1) * C4)
                        for q in (0, 1):
                            qs = slice(half * C4 + q * 2,
                                       half * C4 + q * 2 + 2)
                            nc.tensor.matmul(
                                out=ph[:, q, 0:512],
                                lhsT=wtile["w_id"][:, :],
                                rhs=h0[:, qs, :], start=True, stop=False)
                            nc.tensor.matmul(
                                out=ph[:, q, 0:512],
                                lhsT=wtile["w_id"][:, :],
                                rhs=h1I[:, qs, :], start=False, stop=True)
                        nc.scalar.copy(
                            out=highI[:, hs, :],
                            in_=ph.rearrange("p a w -> p (a w)")[:, 0:1024]
                            .rearrange("p (c w) -> p c w", w=256))

                # ---- store (bf16, row-pair chunks) -------------------------
                nc.sync.dma_start(
                    out=low_d[c0b:c0b + C4, :, :].rearrange(
                        "c (p r) w -> p c r w", r=2),
                    in_=lowhi[:, 0:2 * C4, :].rearrange(
                        "p (c r) w -> p c r w", r=2))
                nc.sync.dma_start(
                    out=high_d[c0b:c0b + C4, :, :].rearrange(
                        "c (p r) w -> p c r w", r=2),
                    in_=highI.rearrange("p (c r) w -> p c r w", r=2))

    nc.compile()
    _NC_CACHE[C] = nc
    return nc


# ----------------------------------------------------------------------------
# host entry points
# ----------------------------------------------------------------------------

_RUNNER = None


def _get_runner():
    """Builds (once) a cached sharded jit executable over the 8 cores."""
    global _RUNNER
    if _RUNNER is not None:
        return _RUNNER

    import jax
    from jax.sharding import Mesh, PartitionSpec, NamedSharding
    from jax.experimental.shard_map import shard_map
    import concourse.mybir as mybir
    from concourse import bass2jax
    from concourse.bass2jax import _bass_exec_p, partition_id_tensor

    bass2jax.install_neuronx_cc_hook()
    nc = build_nc(C_)

    partition_name = nc.partition_id_tensor.name if nc.partition_id_tensor else None
    in_names, out_names, out_avals = [], [], []
    for alloc in nc.m.functions[0].allocations:
        if not isinstance(alloc, mybir.MemoryLocationSet):
            continue
        name = alloc.memorylocations[0].name
        if alloc.kind == "ExternalInput":
            if name != partition_name:
                in_names.append(name)
        elif alloc.kind == "ExternalOutput":
            out_names.append(name)
            out_avals.append(jax.core.ShapedArray(
                tuple(alloc.tensor_shape), mybir.dt.np(alloc.dtype)))
    n_params = len(in_names)
    all_in_names = list(in_names) + list(out_names)
    if partition_name is not None:
        all_in_names.append(partition_name)

    def _body(*args):
        operands = list(args)
        if partition_name is not None:
            operands.append(partition_id_tensor())
        return tuple(_bass_exec_p.bind(
            *operands,
            out_avals=tuple(out_avals),
            in_names=tuple(all_in_names),
            out_names=tuple(out_names),
            lowering_input_output_aliases=(),
            sim_require_finite=True,
            sim_require_nnan=True,
            nc=nc,
        ))

    devices = jax.devices()[:NCORES]
    mesh = Mesh(np.asarray(devices), ("core",))
    n_in = n_params + len(out_names)
    sharded = jax.jit(shard_map(
        _body, mesh=mesh,
        in_specs=(PartitionSpec("core"),) * n_in,
        out_specs=(PartitionSpec("core"),) * len(out_names),
        check_rep=False))

    shard0 = NamedSharding(mesh, PartitionSpec("core"))
    wt = _weights()
    static = {}
    for name in in_names:
        if name == "x":
            continue
        arr = np.concatenate([wt[name]] * NCORES, axis=0)
        static[name] = jax.device_put(arr, shard0)
    for name, aval in zip(out_names, out_avals):
        z = np.zeros((aval.shape[0] * NCORES,) + tuple(aval.shape[1:]),
                     dtype=aval.dtype)
        static[name] = jax.device_put(z, shard0)

    def run(x_global):
        ops = []
        for name in in_names:
            ops.append(x_global if name == "x" else static[name])
        for name in out_names:
            ops.append(static[name])
        outs = sharded(*ops)
        return dict(zip(out_names, outs))

    _RUNNER = (run, shard0)
    return _RUNNER


def _run_device(x, trace=False):
    """x: [8, 64, 256, 256] fp32. Returns (low, high, results_obj)."""
    if trace:
        import shutil
        from concourse import bass_utils
        nc = build_nc(C_)
        wt = _weights()
        in_maps = [dict(wt, x=np.ascontiguousarray(x[b])) for b in range(NCORES)]
        shutil.rmtree("/tmp/bass_trace", ignore_errors=True)
        import os
        os.makedirs("/tmp/bass_trace", exist_ok=True)
        res = bass_utils.run_bass_kernel_spmd(
            nc, in_maps, core_ids=list(range(NCORES)), trace=True,
            tmpdir="/tmp/bass_trace")
        low = np.stack([np.asarray(res.results[b]["low"]) for b in range(NCORES)])
        high = np.stack([np.asarray(res.results[b]["high"]) for b in range(NCORES)])
        return low.astype(np.float32), high.astype(np.float32), res

    run, _ = _get_runner()
    outs = run(np.ascontiguousarray(x).reshape(B_ * C_, H_, W_))
    low = np.asarray(outs["low"]).reshape(B_, C_, H_, W_).astype(np.float32)
    high = np.asarray(outs["high"]).reshape(B_, C_, H_, W_).astype(np.float32)
    return low, high, None


def _fallback(x, level):
    """Numpy port of the reference for unexpected shapes/levels."""
    xl = x.astype(np.float64)
    Bb, Cc, H, W = xl.shape
    low = xl
    high = np.zeros_like(xl)

    def up(a, n_r, n_c):
        Mr = _resize_matrix(a.shape[-2], n_r)
        Mc = _resize_matrix(a.shape[-1], n_c)
        return np.einsum("ij,...jk,lk->...il", Mr, a, Mc)

    for lv in range(level):
        stride = 2 ** lv
        if H // stride < 2 or W // stride < 2:
            break
        x00 = low[..., 0:H - 1:stride, 0:W - 1:stride]
        x01 = low[..., 0:H - 1:stride, 1:W:stride]
        x10 = low[..., 1:H:stride, 0:W - 1:stride]
        x11 = low[..., 1:H:stride, 1:W:stride]
        ll = (x00 + x01 + x10 + x11) * 0.25
        lh = (x00 + x01 - x10 - x11) * 0.25
        hl = (x00 - x01 + x10 - x11) * 0.25
        hh = (x00 - x01 - x10 + x11) * 0.25
        ch = np.abs(lh) + np.abs(hl) + np.abs(hh)
        high = high + up(ch, H, W)
        low = up(ll, H, W)
    if level > 0:
        high = high / level
    return low.astype(np.float32), high.astype(np.float32)


def kernel(x, level):
    x = np.asarray(x, dtype=np.float32)
    level = int(level)
    if level != 2 or x.shape != (B_, C_, H_, W_):
        return _fallback(x, level)
    low, high, _ = _run_device(x)
    return low, high
